# revision 34
# baseline (speedup 1.0000x reference)
"""CrossScaleSelectiveScan Trainium2 Bass kernel.

Sharding: data-parallel over batch B=8 -> one batch per NeuronCore.
Per core: bilinear resizes folded into 1x1-conv matmuls (separable
tap kernels as strided-view matmul accumulation, exact edges via
scaled-identity corrections), two 64-step GRU scans run as independent
latency-chains (vertical + horizontal), then gating + output projection
+ residual. All matmuls bf16 with fp32 PSUM accumulation; gate biases
ride the ACT bias / scalar_tensor_tensor scalar slots so no bias
matmuls or extra adds are needed.
"""
import numpy as np
import ml_dtypes
from contextlib import ExitStack

import concourse.bacc as bacc
import concourse.bass as bass
import concourse.mybir as mybir
import concourse.tile as tile
from concourse.bass_utils import run_bass_kernel_spmd

BF = mybir.dt.bfloat16
F32 = mybir.dt.float32
AF = mybir.ActivationFunctionType
ALU = mybir.AluOpType
NP_BF16 = ml_dtypes.bfloat16

C = 128
H = W = 64
T = 64
PX = H * W          # 4096
HL = WL = 128       # l spatial
HS = WS = 32        # s spatial
NT = 27             # weight tiles in bundle

# weight-bundle tile indices
WL18, WL38, WL37, WL17 = 0, 1, 2, 3
ID18, ID38, ID356, ID156, ID34, ID14 = 4, 5, 6, 7, 8, 9
WS34, WS14 = 10, 11
WM = 12
WIH_H, WHH_H = 13, 16   # +0 r, +1 z, +2 n
WIH_W, WHH_W = 19, 22
GW, PO = 25, 26

# bias columns
B_SHIFT_IN, B_GATE, B_SHIFT_OUT = 0, 1, 2
B_R_H, B_Z_H, B_HHN_H, B_IHN_H = 3, 4, 5, 6
B_R_W, B_Z_W, B_HHN_W, B_IHN_W = 7, 8, 9, 10
NB = 11


def _prep_shared(inp):
    """Build the per-core weight bundle (identical on every core)."""
    f = np.float32
    scale_i = inp['proj_in_scale'].astype(f)
    w_in = inp['proj_in_w'].astype(f) * scale_i[:, None]
    Wl, Wm, Ws = w_in[:, :C], w_in[:, C:2 * C], w_in[:, 2 * C:]
    eye = np.eye(C, dtype=f)

    tiles = [None] * NT
    for idx, k in ((WL18, 1 / 8), (WL38, 3 / 8), (WL37, 3 / 7), (WL17, 1 / 7)):
        tiles[idx] = (k * Wl).T
    for idx, k in ((ID18, 1 / 8), (ID38, 3 / 8), (ID356, 3 / 56),
                   (ID156, 1 / 56), (ID34, 3 / 4), (ID14, 1 / 4)):
        tiles[idx] = k * eye
    for idx, k in ((WS34, 3 / 4), (WS14, 1 / 4)):
        tiles[idx] = (k * Ws).T
    tiles[WM] = Wm.T
    for base, wname in ((WIH_H, 'wih_h'), (WHH_H, 'whh_h'),
                        (WIH_W, 'wih_w'), (WHH_W, 'whh_w')):
        wmat = inp[wname].astype(f)                          # [3C, C]
        for g in range(3):
            tiles[base + g] = wmat[g * C:(g + 1) * C, :].T
    tiles[GW] = inp['gate_w'].astype(f).T
    tiles[PO] = (inp['proj_out_w'].astype(f)
                 * inp['proj_out_scale'].astype(f)[:, None]).T
    wb = np.concatenate(tiles, axis=1).astype(NP_BF16)       # [128, NT*128]

    bih_h, bhh_h = inp['bih_h'].astype(f), inp['bhh_h'].astype(f)
    bih_w, bhh_w = inp['bih_w'].astype(f), inp['bhh_w'].astype(f)
    bias = np.zeros((C, NB), f)
    bias[:, B_SHIFT_IN] = inp['proj_in_shift'].astype(f)
    bias[:, B_GATE] = inp['gate_b'].astype(f)
    bias[:, B_SHIFT_OUT] = inp['proj_out_shift'].astype(f)
    bias[:, B_R_H] = bih_h[:C] + bhh_h[:C]
    bias[:, B_Z_H] = bih_h[C:2 * C] + bhh_h[C:2 * C]
    bias[:, B_HHN_H] = bhh_h[2 * C:]
    bias[:, B_IHN_H] = bih_h[2 * C:]
    bias[:, B_R_W] = bih_w[:C] + bhh_w[:C]
    bias[:, B_Z_W] = bih_w[C:2 * C] + bhh_w[C:2 * C]
    bias[:, B_HHN_W] = bhh_w[2 * C:]
    bias[:, B_IHN_W] = bih_w[2 * C:]
    return wb, bias


def build_nc(loop_n=1):
    nc = bacc.Bacc("TRN2", target_bir_lowering=False)
    l_d = nc.dram_tensor("l", [C, HL * WL], F32, kind="ExternalInput")
    m_d = nc.dram_tensor("m", [C, PX], F32, kind="ExternalInput")
    s_d = nc.dram_tensor("s", [C, HS * WS], F32, kind="ExternalInput")
    wb_d = nc.dram_tensor("wb", [C, NT * C], BF, kind="ExternalInput")
    bias_d = nc.dram_tensor("bias", [C, NB], F32, kind="ExternalInput")
    out_d = nc.dram_tensor("out", [C, PX], F32, kind="ExternalOutput")

    with tile.TileContext(nc) as tc, ExitStack() as ctx:
        big = ctx.enter_context(tc.tile_pool(name="big", bufs=1))
        l_ts = [big.tile([C, 32 * WL], BF, name=f"lt{i}", tag=f"l{i}")
                for i in range(4)]
        m_sb = big.tile([C, PX], BF, tag="m")
        s_sb = big.tile([C, HS * WS], BF, tag="s")
        wb = big.tile([C, NT * C], BF, tag="wb")
        bias = big.tile([C, NB], F32, tag="bias")
        o1 = big.tile([C, HL * W], BF, tag="o1")      # (h=128, w'=64)
        o1s = big.tile([C, HS * W], BF, tag="o1s")    # (hs=32, w'=64)
        x_sb = big.tile([C, PX], BF, tag="x")
        xc = [big.tile([C, 512], BF, name=f"xc{i}", tag=f"xc{i}")
              for i in range(8)]
        gxn_t = {('h', c): big.tile([C, 512], BF, name=f"gxnh{c}", tag=f"gxnh{c}")
                 for c in range(8)}
        gxn_t.update({('w', c): big.tile([C, 512], BF, name=f"gxnw{c}", tag=f"gxnw{c}")
                      for c in range(8)})
        oh = big.tile([C, PX], BF, tag="oh")          # (h, w)
        ow_ts = [big.tile([C, 512], BF, name=f"owt{i}", tag=f"owt{i}")
                 for i in range(8)]                   # (w, h) in 8 w-chunks
        h0 = big.tile([C, 64], BF, tag="h0")
        scd = big.tile([C, PX], BF, tag="scd")
        out_ts = [big.tile([C, 1024], F32, name=f"outsb{i}", tag=f"outsb{i}")
                  for i in range(4)]

        nc.sync.dma_start(wb[:], wb_d[:])
        nc.sync.dma_start(bias[:], bias_d[:])
        nc.vector.memset(h0[:], 0.0)

        def wt(i):
            return wb[:, i * C:(i + 1) * C]

        def bcol(i):
            return bias[:, i:i + 1]

        sv = s_sb[:].rearrange("p (h w) -> p h w", h=HS)
        o1v = o1[:].rearrange("p (h w) -> p h w", h=HL)
        o1sv = o1s[:].rearrange("p (h w) -> p h w", h=HS)
        xv = x_sb[:].rearrange("p (h w) -> p h w", h=H)
        mm = nc.tensor.matmul

        for _it in range(loop_n):
            # ---- loads (SWDGE casts f32 -> bf16 in flight)
            nc.gpsimd.dma_start(m_sb[:], m_d[:])
            nc.gpsimd.dma_start(s_sb[:], s_d[:])
            for i in range(4):
                nc.gpsimd.dma_start(l_ts[i][:], l_d[:, i * 4096:(i + 1) * 4096])
            lvs = [t[:].rearrange("p (h w) -> p h w", h=32) for t in l_ts]

            # ===== P1a: l W-downsample fused with conv -> o1
            with tc.tile_pool(name="ps1", bufs=2, space="PSUM") as ps1, \
                 tc.tile_pool(name="pse", bufs=1, space="PSUM") as pse:
                for k in range(16):                   # chunks of 8 h-rows
                    lt = lvs[k // 4]
                    r0 = 8 * (k % 4)
                    p = ps1.tile([C, 512], F32, tag="o1p")
                    pvv = p[:].rearrange("p (h w) -> p h w", h=8)
                    rows = lt[:, r0:r0 + 8, :]
                    mm(pvv[:, :, :], wt(WL38), rows[:, :, 0:128:2], start=True, stop=False, skip_group_check=True)
                    mm(pvv[:, :, :], wt(WL38), rows[:, :, 1:128:2], start=False, stop=False, skip_group_check=True)
                    mm(pvv[:, :, 1:64], wt(WL18), rows[:, :, 1:126:2], start=False, stop=False, skip_group_check=True)
                    mm(pvv[:, :, 0:63], wt(WL18), rows[:, :, 2:127:2], start=False, stop=True, skip_group_check=True)
                    eng = nc.vector if (k % 2 == 0) else nc.scalar
                    if eng is nc.vector:
                        eng.tensor_copy(o1[:, k * 512:(k + 1) * 512], p[:])
                    else:
                        eng.activation(o1[:, k * 512:(k + 1) * 512], p[:], AF.Copy)

                # exact edge columns w'=0 / w'=63
                ep = pse.tile([C, 256], F32, tag="ep")
                epv = ep[:].rearrange("p (e h) -> p e h", e=2)
                first = True
                for lti in range(4):
                    for (wcol, widx) in ((0, WL37), (1, WL37), (2, WL17)):
                        mm(epv[:, 0, 32 * lti:32 * lti + 32], wt(widx),
                           lvs[lti][:, :, wcol], start=first, stop=False, skip_group_check=True)
                        first = False
                for lti in range(4):
                    for i, (wcol, widx) in enumerate(((125, WL17), (126, WL37), (127, WL37))):
                        mm(epv[:, 1, 32 * lti:32 * lti + 32], wt(widx),
                           lvs[lti][:, :, wcol], start=False,
                           stop=(lti == 3 and i == 2), skip_group_check=True)
                nc.vector.tensor_copy(o1v[:, :, 0], epv[:, 0, :])
                nc.vector.tensor_copy(o1v[:, :, 63], epv[:, 1, :])

            # ===== P1b: s W-upsample fused with conv -> o1s
            with tc.tile_pool(name="pss", bufs=2, space="PSUM") as pss:
                for k in range(4):
                    r0 = 8 * k
                    p = pss.tile([C, 512], F32, tag="o1sp")
                    pvv = p[:].rearrange("p (h w) -> p h w", h=8)
                    srows = sv[:, r0:r0 + 8, :]
                    mm(pvv[:, :, 0:64:2], wt(WS34), srows[:, :, 0:32], start=True, stop=False, skip_group_check=True)
                    mm(pvv[:, :, 1:64:2], wt(WS34), srows[:, :, 0:32], start=False, stop=False, skip_group_check=True)
                    mm(pvv[:, :, 2:64:2], wt(WS14), srows[:, :, 0:31], start=False, stop=False, skip_group_check=True)
                    mm(pvv[:, :, 1:63:2], wt(WS14), srows[:, :, 1:32], start=False, stop=False, skip_group_check=True)
                    mm(pvv[:, :, 0], wt(WS14), srows[:, :, 0], start=False, stop=False, skip_group_check=True)
                    mm(pvv[:, :, 63], wt(WS14), srows[:, :, 31], start=False, stop=True, skip_group_check=True)
                    eng = nc.vector if (k % 2 == 0) else nc.scalar
                    if eng is nc.vector:
                        eng.tensor_copy(o1s[:, k * 512:(k + 1) * 512], p[:])
                    else:
                        eng.activation(o1s[:, k * 512:(k + 1) * 512], p[:], AF.Copy)

            # ===== P1c + P2 merged: x chunks stream into the H scan,
            # W scan trails by LEAD steps (it needs the full x).
            # P3 (gate + proj-out + residual) is pipelined INTO the u-loop:
            # chunk c streams right behind W step 8c+7 so almost no tail
            # remains after the scans drain.
            with tc.tile_pool(name="psn", bufs=1, space="PSUM") as psn, \
                 tc.tile_pool(name="psH", bufs=1, space="PSUM") as psH, \
                 tc.tile_pool(name="psW", bufs=1, space="PSUM") as psW, \
                 tc.tile_pool(name="sc", bufs=4) as sc, \
                 tc.tile_pool(name="psx", bufs=1, space="PSUM") as psx, \
                 tc.tile_pool(name="p3", bufs=2) as p3:

                def x_chunk(c):
                    hp0 = 8 * c
                    p = psx.tile([C, 512], F32, tag="xp", name=f"xp{c}")
                    pvv = p[:].rearrange("p (h w) -> p h w", h=8)
                    mm(p[:], wt(WM), m_sb[:, c * 512:(c + 1) * 512], start=True, stop=False, skip_group_check=True)
                    mm(pvv[:, :, :], wt(ID38), o1v[:, 2 * hp0:2 * hp0 + 16:2, :], start=False, stop=False, skip_group_check=True)
                    mm(pvv[:, :, :], wt(ID38), o1v[:, 2 * hp0 + 1:2 * hp0 + 16:2, :], start=False, stop=False, skip_group_check=True)
                    if c == 0:
                        mm(pvv[:, 1:8, :], wt(ID18), o1v[:, 1:15:2, :], start=False, stop=False, skip_group_check=True)
                    else:
                        mm(pvv[:, :, :], wt(ID18), o1v[:, 2 * hp0 - 1:2 * hp0 + 15:2, :], start=False, stop=False, skip_group_check=True)
                    if c == 7:
                        mm(pvv[:, 0:7, :], wt(ID18), o1v[:, 2 * hp0 + 2:2 * hp0 + 16:2, :], start=False, stop=False, skip_group_check=True)
                    else:
                        mm(pvv[:, :, :], wt(ID18), o1v[:, 2 * hp0 + 2:2 * hp0 + 18:2, :], start=False, stop=False, skip_group_check=True)
                    if c == 0:
                        mm(pvv[:, 0, :], wt(ID356), o1v[:, 0, :], start=False, stop=False, skip_group_check=True)
                        mm(pvv[:, 0, :], wt(ID356), o1v[:, 1, :], start=False, stop=False, skip_group_check=True)
                        mm(pvv[:, 0, :], wt(ID156), o1v[:, 2, :], start=False, stop=False, skip_group_check=True)
                    if c == 7:
                        mm(pvv[:, 7, :], wt(ID156), o1v[:, 125, :], start=False, stop=False, skip_group_check=True)
                        mm(pvv[:, 7, :], wt(ID356), o1v[:, 126, :], start=False, stop=False, skip_group_check=True)
                        mm(pvv[:, 7, :], wt(ID356), o1v[:, 127, :], start=False, stop=False, skip_group_check=True)
                    p0 = 4 * c
                    mm(pvv[:, 0:8:2, :], wt(ID34), o1sv[:, p0:p0 + 4, :], start=False, stop=False, skip_group_check=True)
                    mm(pvv[:, 1:8:2, :], wt(ID34), o1sv[:, p0:p0 + 4, :], start=False, stop=False, skip_group_check=True)
                    if c == 0:
                        mm(pvv[:, 2:8:2, :], wt(ID14), o1sv[:, 0:3, :], start=False, stop=False, skip_group_check=True)
                        mm(pvv[:, 0, :], wt(ID14), o1sv[:, 0, :], start=False, stop=False, skip_group_check=True)
                    else:
                        mm(pvv[:, 0:8:2, :], wt(ID14), o1sv[:, p0 - 1:p0 + 3, :], start=False, stop=False, skip_group_check=True)
                    if c == 7:
                        mm(pvv[:, 1:7:2, :], wt(ID14), o1sv[:, 29:32, :], start=False, stop=False, skip_group_check=True)
                        mm(pvv[:, 7, :], wt(ID14), o1sv[:, 31, :], start=False, stop=True, skip_group_check=True)
                    else:
                        mm(pvv[:, 1:8:2, :], wt(ID14), o1sv[:, p0 + 1:p0 + 5, :], start=False, stop=(c != 0), skip_group_check=True)
                    nc.scalar.activation(xc[c][:], p[:],
                                         AF.Relu, bias=bcol(B_SHIFT_IN))
                    nc.vector.tensor_copy(x_sb[:, c * 512:(c + 1) * 512], xc[c][:])
                    # gxn for the H scan streams right behind each x chunk
                    pg = psn.tile([C, 512], F32, tag="gxnp", name=f"gxnp{c}")
                    mm(pg[:], wt(WIH_H + 2), xc[c][:],
                       start=True, stop=True, skip_group_check=True)
                    nc.vector.tensor_scalar(gxn_t[('h', c)][:], pg[:],
                                            bcol(B_IHN_H), None, ALU.add)

                def gxn_w_chunk(c):
                    p = psn.tile([C, 512], F32, tag="gxnp", name=f"gxnw{c}")
                    rhs = xv[:, :, 8 * c:8 * c + 8].transpose([0, 2, 1])
                    mm(p[:].rearrange("p (w h) -> p w h", w=8), wt(WIH_W + 2), rhs,
                       start=True, stop=True, skip_group_check=True)
                    # bias-add in [C,256] halves (GPSIMD has no PSUM port, so
                    # these stay on DVE; halves bound the chain-delay spill)
                    for hh in range(2):
                        nc.vector.tensor_scalar(
                            gxn_t[('w', c)][:, hh * 256:(hh + 1) * 256],
                            p[:, hh * 256:(hh + 1) * 256],
                            bcol(B_IHN_W), None, ALU.add)

                scans = {
                    'h': (psH, 'h', WIH_H, WHH_H, B_R_H, B_Z_H, B_HHN_H),
                    'w': (psW, 'w', WIH_W, WHH_W, B_R_W, B_Z_W, B_HHN_W),
                }

                def hbuf(sname, t):
                    if sname == 'h':
                        return oh[:, t * 64:(t + 1) * 64]
                    return ow_ts[t // 8][:, (t % 8) * 64:(t % 8) * 64 + 64]
                Pcur = {}
                prev = {'h': None, 'w': None}

                def scan_mms(sname, t):
                    pool, buf, wih, whh = scans[sname][:4]
                    Pr_t = pool.tile([C, 64], F32, tag=f"Pr{sname}", name=f"Pr{sname}{t}")
                    Pz_t = pool.tile([C, 64], F32, tag=f"Pz{sname}", name=f"Pz{sname}{t}")
                    Pn_t = pool.tile([C, 64], F32, tag=f"Pn{sname}", name=f"Pn{sname}{t}")
                    Pr, Pz, Pn = Pr_t[:], Pz_t[:], Pn_t[:]
                    xt = (xc[t // 8][:, (t % 8) * 64:(t % 8) * 64 + 64]
                          if sname == 'h' else xv[:, :, t])
                    mm(Pr, wt(wih + 0), xt, start=True, stop=False, skip_group_check=True)
                    mm(Pz, wt(wih + 1), xt, start=True, stop=False, skip_group_check=True)
                    if prev[sname] is None:
                        mm(Pr, wt(whh + 0), h0[:], start=False, stop=True, skip_group_check=True)
                        mm(Pn, wt(whh + 2), h0[:], start=True, stop=True, skip_group_check=True)
                        mm(Pz, wt(whh + 1), h0[:], start=False, stop=True, skip_group_check=True)
                    else:
                        # h' = t1 + t2, and whh@h' = whh@t2 + whh@t1: the t2
                        # half issues early (t2 is ready before tanh), only
                        # the t1 half waits on the tanh chain. Separate PSUM
                        # banks per gate keep sigma_r off the z/n writers.
                        t1p, t2p = prev[sname]
                        mm(Pr, wt(whh + 0), t2p[:], start=False, stop=False, skip_group_check=True)
                        mm(Pn, wt(whh + 2), t2p[:], start=True, stop=False, skip_group_check=True)
                        mm(Pz, wt(whh + 1), t2p[:], start=False, stop=False, skip_group_check=True)
                        mm(Pr, wt(whh + 0), t1p[:], start=False, stop=True, skip_group_check=True)
                        mm(Pn, wt(whh + 2), t1p[:], start=False, stop=True, skip_group_check=True)
                        mm(Pz, wt(whh + 1), t1p[:], start=False, stop=True, skip_group_check=True)
                    Pcur[sname] = (Pr, Pz, Pn)

                def scan_gates(sname, t):
                    pool, buf, wih, whh, br, bz, bhhn = scans[sname]
                    Pr, Pz, Pn = Pcur[sname]
                    hp = h0[:] if t == 0 else hbuf(sname, t - 1)
                    r = sc.tile([C, 64], BF, tag=f"r{sname}", name=f"r{sname}{t}")
                    nc.scalar.activation(r[:], Pr, AF.Sigmoid, bias=bcol(br))
                    q = sc.tile([C, 64], BF, tag=f"q{sname}", name=f"q{sname}{t}")
                    nc.vector.scalar_tensor_tensor(q[:], Pn,
                                                   bcol(bhhn), r[:],
                                                   ALU.add, ALU.mult)
                    nin = sc.tile([C, 64], BF, tag=f"nin{sname}", name=f"nin{sname}{t}")
                    nc.vector.tensor_add(nin[:], q[:],
                                         gxn_t[(sname, t // 8)][:, (t % 8) * 64:(t % 8) * 64 + 64])
                    n = sc.tile([C, 64], BF, tag=f"n{sname}", name=f"n{sname}{t}")
                    nc.scalar.activation(n[:], nin[:], AF.Tanh)
                    # sigma_z emits after tanh: it is off the critical chain,
                    # and ahead of tanh in the strict ACT FIFO it can block
                    # the chain when Pz's last matmul lands late.
                    z = sc.tile([C, 64], BF, tag=f"z{sname}", name=f"z{sname}{t}")
                    nc.scalar.activation(z[:], Pz, AF.Sigmoid, bias=bcol(bz))
                    zc = sc.tile([C, 64], BF, tag=f"zc{sname}", name=f"zc{sname}{t}")
                    nc.gpsimd.tensor_scalar(zc[:], z[:], -1.0, 1.0,
                                            ALU.mult, ALU.add)
                    t2 = sc.tile([C, 64], BF, tag=f"t2{sname}", name=f"t2{sname}{t}")
                    nc.gpsimd.tensor_mul(t2[:], z[:], hp)
                    t1 = sc.tile([C, 64], BF, tag=f"t1{sname}", name=f"t1{sname}{t}")
                    nc.vector.tensor_mul(t1[:], n[:], zc[:])
                    nc.vector.tensor_add(hbuf(sname, t), t1[:], t2[:])
                    prev[sname] = (t1, t2)

                # ---- P3 stages, issued piecewise inside the u-loop.
                # Chunk c covers w-cols 8c..8c+7 (w,h)-major; ready once W
                # step 8c+7's gates ran. Stages spread over consecutive u's
                # so each engine sees at most one slack-sized P3 op per u.
                # The two P3 matmuls share psx's single PSUM bank (free
                # after the head) — WAR serializes gp->sigma_g->op->relu,
                # which is fine off-chain.
                ohv = oh[:].rearrange("p (h w) -> p h w", h=H)

                def p3_stage0(c):
                    # scd = ow + oh^T in [C,256] halves (4 w-cols each) on Pool
                    for hh in range(2):
                        h2 = slice(c * 512 + hh * 256, c * 512 + (hh + 1) * 256)
                        oh_view = ohv[:, :, 8 * c + 4 * hh:8 * c + 4 * hh + 4] \
                            .transpose([0, 2, 1])
                        nc.gpsimd.tensor_add(scd[:, h2],
                                             ow_ts[c][:, hh * 256:(hh + 1) * 256],
                                             oh_view)

                def p3_stage1(c):
                    sl = slice(c * 512, (c + 1) * 512)
                    gp = psx.tile([C, 512], F32, tag="xp", name=f"gp{c}")
                    g = p3.tile([C, 512], BF, tag="g", name=f"g{c}")
                    for hh in range(2):
                        h2 = slice(hh * 256, (hh + 1) * 256)
                        mm(gp[:, h2], wt(GW), scd[:, sl][:, h2],
                           start=True, stop=True, skip_group_check=True)
                        nc.scalar.activation(g[:, h2], gp[:, h2], AF.Sigmoid,
                                             bias=bcol(B_GATE))
                    return g

                def p3_stage2(c, g):
                    sl = slice(c * 512, (c + 1) * 512)
                    gated = p3.tile([C, 512], BF, tag="gated", name=f"gated{c}")
                    op = psx.tile([C, 512], F32, tag="xp", name=f"op{c}")
                    for hh in range(2):
                        h2 = slice(hh * 256, (hh + 1) * 256)
                        nc.vector.tensor_mul(gated[:, h2], scd[:, sl][:, h2], g[:, h2])
                        mm(op[:, h2], wt(PO), gated[:, h2],
                           start=True, stop=True, skip_group_check=True)
                    return op

                def p3_stage3(c, op):
                    y = p3.tile([C, 512], BF, tag="y", name=f"y{c}")
                    for hh in range(2):
                        h2 = slice(hh * 256, (hh + 1) * 256)
                        nc.vector.tensor_scalar(y[:, h2], op[:, h2],
                                                bcol(B_SHIFT_OUT), 0.0,
                                                ALU.add, ALU.max)
                    return y

                def p3_stage4(c, y):
                    for hh in range(2):
                        h2 = slice(hh * 256, (hh + 1) * 256)
                        x_view = xv[:, :, 8 * c + 4 * hh:8 * c + 4 * hh + 4] \
                            .transpose([0, 2, 1])
                        nc.vector.tensor_add(
                            out_ts[c // 2][:, (c % 2) * 512 + hh * 256:
                                           (c % 2) * 512 + (hh + 1) * 256],
                            y[:, h2], x_view)
                    if c % 2 == 1:
                        nc.sync.dma_start(out_d[:, (c - 1) * 512:(c + 1) * 512],
                                          out_ts[c // 2][:])

                # x chunks + the first 16 H steps interleave so the H chain
                # starts as soon as x chunk 0 exists instead of after P1c.
                for c in range(8):
                    x_chunk(c)
                    for t in (2 * c, 2 * c + 1):
                        scan_mms('h', t)
                        scan_gates('h', t)

                # P3 needs the FULL H-scan (oh rows 0..63) plus ow chunk c,
                # so its earliest issue point is u=T (after H step 63 is
                # issued). Diagonal pipeline through the W-tail u's; chunk 7
                # drains post-loop.
                LEAD = 16
                p3_sched = {}          # u -> list of (stage_idx, chunk)
                for c3 in range(8):
                    k0 = {6: 7, 7: 15}.get(c3, 2 * c3)
                    for s in range(5):
                        uu = T + k0 + s
                        if uu < T + LEAD:
                            p3_sched.setdefault(uu, []).append((s, c3))

                p3g = {}
                gxn_w_chunk(0)
                for u in range(LEAD, T + LEAD):
                    tH, tW = u, u - LEAD
                    if tH < T:
                        scan_mms('h', tH)
                    scan_mms('w', tW)
                    if tH < T:
                        scan_gates('h', tH)
                    scan_gates('w', tW)
                    if tW % 8 == 6 and tW < 56:
                        gxn_w_chunk(tW // 8 + 1)
                    for (s, c3) in p3_sched.get(u, []):
                        if s == 0:
                            p3_stage0(c3)
                        elif s == 1:
                            p3g[c3] = p3_stage1(c3)
                        elif s == 2:
                            p3g[c3] = p3_stage2(c3, p3g[c3])
                        elif s == 3:
                            p3g[c3] = p3_stage3(c3, p3g[c3])
                        else:
                            p3_stage4(c3, p3g.pop(c3))
                # drain chunk 7 (stage0 ran at the last u)
                g7 = p3_stage1(7)
                o7 = p3_stage2(7, g7)
                y7 = p3_stage3(7, o7)
                p3_stage4(7, y7)

    nc.finalize()
    return nc


_NC_CACHE = {}


def kernel(**inputs):
    inputs = {k: np.asarray(v) for k, v in inputs.items()}
    B = inputs['l'].shape[0]
    wb, bias = _prep_shared(inputs)
    if 'nc' not in _NC_CACHE:
        _NC_CACHE['nc'] = build_nc()
    nc = _NC_CACHE['nc']
    in_maps = []
    for b in range(B):
        in_maps.append({
            'l': inputs['l'][b].reshape(C, -1).astype(np.float32),
            'm': inputs['m'][b].reshape(C, -1).astype(np.float32),
            's': inputs['s'][b].reshape(C, -1).astype(np.float32),
            'wb': wb, 'bias': bias,
        })
    res = run_bass_kernel_spmd(nc, in_maps, core_ids=list(range(B)))
    # device output is (w,h)-major; unpermute on host
    out = np.stack([res.results[b]['out'].reshape(C, W, H).transpose(0, 2, 1)
                    for b in range(B)], 0)
    return out.astype(np.float32)

